# revision 9
# baseline (speedup 1.0000x reference)
"""Trainium2 Bass kernel for nn_BuildCorrelation.

Math (per batch b):
    Q = X Wq^T + bq; K = X Wk^T + bk; V = X Wv^T + bv      [N=1024, E=32]
    S = Q K^T / 32 ; A = softmax(S, axis=-1); F = A V
    corr = rowwise-corrcoef Gram of F, clipped to [-1, 1]

Key identity used: corr is invariant to per-row scaling of F.  With
E_xp = exp(S/32) (no softmax normalization, no max subtraction — S/32 is
tiny for this data distribution) and G = E_xp @ V, the rows of G are
positive multiples of the rows of F, so
    corr = clip(U U^T),  U[n,:] = (G[n,:] - mean G[n,:]) / ||G[n,:] - mean||.

Layout strategy per batch (all matmuls contract over the partition dim):
    X^T [64, N]  (8 PE transposes of DMA'd [128, 64] tiles)
    QKV^T = W^T-stacked lhsT [64, 96] @ X^T  -> [96, N] (+bias via ACT copy)
    S'_tile = (K^T chunk)^T @ Q^T chunk = S^T tile [128 m, 512 n]
    E^T = exp(S'/32)  (ACT, writes float32r)
    G^T += V_chunk^T-free lhsT [128 m, 32 e] @ E^T  (PSUM accumulation)
    G^T -> G natural (PE transposes), center+normalize rows (DVE/ACT)
    U natural -> U^T (PE transposes)
    corr tile = (U^T chunk)^T @ U^T chunk -> clip -> DMA out

Batch dim (64) is sharded across the 8 cores, params replicated.
"""

import sys

if "/opt/trn_rl_repo" not in sys.path:
    sys.path.insert(0, "/opt/trn_rl_repo")

import numpy as np

import concourse.bass as bass
import concourse.tile as tile
from concourse import mybir
from concourse.bass_utils import run_bass_kernel_spmd

F32 = mybir.dt.float32
F32R = mybir.dt.float32r
AF = mybir.ActivationFunctionType
ALU = mybir.AluOpType

N_CORES = 8
B = 64
N = 1024
D = 64
E = 32
P = 128
FREE = 512
NCHUNK = N // P  # 8
NF = N // FREE  # 2
B_PER_CORE = B // N_CORES  # 8


def split_multi_waits(nc):
    """The walrus build here accepts at most ONE sync wait per instruction
    ("Too many sync wait commands").  Hoist extra waits onto same-engine
    nops inserted immediately before the over-subscribed instruction."""
    ctr = 0
    for f in nc.m.functions:
        for bb in f.blocks:
            out = []
            for inst in bb.instructions:
                si = inst.sync_info
                if si is not None and si.on_wait and len(si.on_wait) > 1:
                    waits = list(si.on_wait)
                    for w in waits[:-1]:
                        ctr += 1
                        out.append(
                            mybir.InstNoOp(
                                name=f"I-ws{ctr}",
                                engine=inst.engine,
                                sync_info=mybir.SyncInfo(on_wait=[w], on_update=[]),
                            )
                        )
                    inst.sync_info = mybir.SyncInfo(
                        on_wait=[waits[-1]], on_update=list(si.on_update)
                    )
                out.append(inst)
            bb.instructions = out


def build_nc(b_per_core=B_PER_CORE):
    nc = bass.Bass("TRN2", target_bir_lowering=False)
    X = nc.dram_tensor("X", [b_per_core, N, D], F32, kind="ExternalInput")
    WQKV = nc.dram_tensor("WQKV", [D, 3 * E], F32, kind="ExternalInput")
    BIA = nc.dram_tensor("BIA", [3 * E, 1], F32, kind="ExternalInput")
    IDN = nc.dram_tensor("IDN", [P, P], F32, kind="ExternalInput")
    OUT = nc.dram_tensor("OUT", [b_per_core, N, N], F32, kind="ExternalOutput")

    with tile.TileContext(nc) as tc:
        with (
            tc.tile_pool(name="const", bufs=1) as const,
            tc.tile_pool(name="sb", bufs=2) as sb,
            tc.tile_pool(name="xin", bufs=3) as xin,
            tc.tile_pool(name="et", bufs=2) as etp,
            tc.tile_pool(name="ot", bufs=3) as otp,
            tc.tile_pool(name="small", bufs=3) as small,
            tc.tile_pool(name="pt", bufs=2, space="PSUM") as pt,
            tc.tile_pool(name="psum_s", bufs=2, space="PSUM") as ps_s,
            tc.tile_pool(name="psum_c", bufs=2, space="PSUM") as ps_c,
            tc.tile_pool(name="psum_g", bufs=1, space="PSUM") as ps_g,
        ):
            # --- constants (replicated, loaded once) ---
            w_raw = const.tile([D, 3 * E], F32)
            nc.sync.dma_start(out=w_raw, in_=WQKV[:, :])
            wqkv = const.tile([D, 3 * E], F32R)
            nc.vector.tensor_copy(wqkv, w_raw)  # round to f32r
            biases = []
            for j in range(3):
                bt = const.tile([E, 1], F32, tag=f"bias{j}")
                nc.sync.dma_start(out=bt, in_=BIA[j * E : (j + 1) * E, :])
                biases.append(bt)
            idn = const.tile([P, P], F32)
            nc.sync.dma_start(out=idn, in_=IDN[:, :])
            idnr = const.tile([E, E], F32R)
            nc.vector.tensor_copy(idnr, idn[0:E, 0:E])

            for b in range(b_per_core):
                # ---- X^T [64, N] in f32r ----
                xT = sb.tile([D, N], F32R, tag="xT")
                for i in range(NCHUNK):
                    xn = xin.tile([P, D], F32, tag="xn")
                    nc.sync.dma_start(out=xn, in_=X[b, i * P : (i + 1) * P, :])
                    ptile = pt.tile([D, P], F32, tag="t")
                    nc.tensor.transpose(ptile, xn, idn)
                    nc.vector.tensor_copy(xT[:, i * P : (i + 1) * P], ptile)

                # ---- Q^T/K^T/V^T [32, N] = wj.T @ X^T (+bias) in f32r ----
                qT = sb.tile([E, N], F32R, tag="qT")
                kT = sb.tile([E, N], F32R, tag="kT")
                vT = sb.tile([E, N], F32R, tag="vT")
                for j, dst in enumerate([qT, kT, vT]):
                    for h in range(NF):
                        pj = pt.tile([E, FREE], F32, tag="t")
                        nc.tensor.matmul(
                            pj,
                            wqkv[:, j * E : (j + 1) * E],
                            xT[:, h * FREE : (h + 1) * FREE],
                            start=True,
                            stop=True,
                        )
                        nc.scalar.activation(
                            dst[:, h * FREE : (h + 1) * FREE],
                            pj,
                            AF.Identity,
                            bias=biases[j],
                            scale=1.0,
                        )

                # ---- V natural [128, 8, 32] in f32r (PE transposes of V^T) ----
                vn = sb.tile([P, NCHUNK, E], F32R, tag="vn")
                for i in range(NCHUNK):
                    pv = pt.tile([P, E], F32R, tag="t")
                    nc.tensor.transpose(pv, vT[:, i * P : (i + 1) * P], idnr)
                    nc.vector.tensor_copy(vn[:, i, :], pv)

                # ---- S' -> exp -> G^T accumulation ----
                gps = [
                    ps_g.tile([E, FREE], F32, tag=f"g{h}", name=f"gps{h}")
                    for h in range(NF)
                ]
                for h in range(NF):
                    for i in range(NCHUNK):
                        pss = ps_s.tile([P, FREE], F32, tag="s")
                        nc.tensor.matmul(
                            pss,
                            kT[:, i * P : (i + 1) * P],
                            qT[:, h * FREE : (h + 1) * FREE],
                            start=True,
                            stop=True,
                        )
                        et = etp.tile([P, FREE], F32R, tag="et")
                        nc.scalar.activation(et, pss, AF.Exp, scale=1.0 / 32.0)
                        nc.tensor.matmul(
                            gps[h],
                            vn[:, i, :],
                            et,
                            start=(i == 0),
                            stop=(i == NCHUNK - 1),
                        )
                gT = sb.tile([E, N], F32, tag="gT")
                for h in range(NF):
                    nc.vector.tensor_copy(gT[:, h * FREE : (h + 1) * FREE], gps[h])

                # ---- normalize rows of G (natural layout) -> U^T f32r ----
                uT = sb.tile([E, N], F32R, tag="uT")
                for i in range(NCHUNK):
                    pg = pt.tile([P, E], F32, tag="t")
                    nc.tensor.transpose(
                        pg, gT[:, i * P : (i + 1) * P], idn[0:E, 0:E]
                    )
                    gn = small.tile([P, E], F32, tag="gn")
                    nc.vector.tensor_copy(gn, pg)
                    stats = small.tile([P, 6], F32, tag="st")
                    nc.vector.bn_stats(stats, gn)
                    mv = small.tile([P, 2], F32, tag="mv")
                    nc.vector.bn_aggr(mv, stats)
                    sq = small.tile([P, 1], F32, tag="sq")
                    nc.scalar.activation(sq, mv[:, 1:2], AF.Sqrt, scale=float(E))
                    rf = small.tile([P, 1], F32, tag="rf")
                    nc.vector.reciprocal(rf, sq)
                    un = small.tile([P, E], F32, tag="un")
                    nc.vector.tensor_scalar(
                        un, gn, mv[:, 0:1], rf, ALU.subtract, ALU.mult
                    )
                    pu = pt.tile([E, P], F32, tag="t")
                    nc.tensor.transpose(pu, un, idn)
                    nc.vector.tensor_copy(uT[:, i * P : (i + 1) * P], pu)

                # ---- corr = clip(U U^T) -> DRAM ----
                for i in range(NCHUNK):
                    ot = otp.tile([P, N], F32, tag="ot")
                    for h in range(NF):
                        pc = ps_c.tile([P, FREE], F32, tag="c")
                        nc.tensor.matmul(
                            pc,
                            uT[:, i * P : (i + 1) * P],
                            uT[:, h * FREE : (h + 1) * FREE],
                            start=True,
                            stop=True,
                        )
                        nc.vector.tensor_scalar(
                            ot[:, h * FREE : (h + 1) * FREE],
                            pc,
                            1.0,
                            -1.0,
                            ALU.min,
                            ALU.max,
                        )
                    nc.sync.dma_start(
                        out=OUT[b, i * P : (i + 1) * P, :], in_=ot
                    )

    split_multi_waits(nc)
    return nc


_NC_CACHE = {}


def _get_nc(b_per_core):
    if b_per_core not in _NC_CACHE:
        _NC_CACHE[b_per_core] = build_nc(b_per_core)
    return _NC_CACHE[b_per_core]


def make_in_maps(BOLDSignals, Wq, bq, Wk, bk, Wv, bv, n_cores=N_CORES):
    wqkv = np.concatenate([Wq.T, Wk.T, Wv.T], axis=1).astype(np.float32)
    bia = np.concatenate([bq, bk, bv]).astype(np.float32)[:, None]
    idn = np.eye(P, dtype=np.float32)
    b_per_core = BOLDSignals.shape[0] // n_cores
    in_maps = []
    for c in range(n_cores):
        in_maps.append(
            {
                "X": np.ascontiguousarray(
                    BOLDSignals[c * b_per_core : (c + 1) * b_per_core],
                    dtype=np.float32,
                ),
                "WQKV": wqkv,
                "BIA": bia,
                "IDN": idn,
            }
        )
    return in_maps


def kernel(
    BOLDSignals,
    EmptyCorrelations=None,
    Wq=None,
    bq=None,
    Wk=None,
    bk=None,
    Wv=None,
    bv=None,
    **_unused,
):
    BOLDSignals = np.asarray(BOLDSignals, dtype=np.float32)
    nb = BOLDSignals.shape[0]
    assert nb % N_CORES == 0, nb
    b_per_core = nb // N_CORES
    nc = _get_nc(b_per_core)
    in_maps = make_in_maps(BOLDSignals, Wq, bq, Wk, bk, Wv, bv)
    res = run_bass_kernel_spmd(nc, in_maps, core_ids=list(range(N_CORES)))
    return np.concatenate([res.results[c]["OUT"] for c in range(N_CORES)], axis=0)


if __name__ == "__main__":
    rng = np.random.default_rng(0)
    inputs = {
        "BOLDSignals": rng.standard_normal((B, N, D), dtype=np.float32),
        "EmptyCorrelations": np.zeros((B, N, N), dtype=np.float32),
    }
    bound = 1.0 / np.sqrt(D)
    for nm in ["q", "k", "v"]:
        inputs[f"W{nm}"] = rng.uniform(-bound, bound, (E, D)).astype(np.float32)
        inputs[f"b{nm}"] = rng.uniform(-bound, bound, (E,)).astype(np.float32)
    out = kernel(**inputs)
    print("out", out.shape, out.dtype, out.min(), out.max())


# revision 19
# speedup vs baseline: 1.1778x; 1.1778x over previous
"""Trainium2 Bass kernel for nn_BuildCorrelation.

Math (per batch b):
    Q = X Wq^T + bq; K = X Wk^T + bk; V = X Wv^T + bv      [N=1024, E=32]
    S = Q K^T / 32 ; A = softmax(S, axis=-1); F = A V
    corr = rowwise-corrcoef Gram of F, clipped to [-1, 1]

Key identity used: corr is invariant to per-row scaling of F.  With
E_xp = exp(S/32) (no softmax normalization, no max subtraction — S/32 is
tiny for this data distribution) and G = E_xp @ V, the rows of G are
positive multiples of the rows of F, so
    corr = clip(U U^T),  U[n,:] = (G[n,:] - mean G[n,:]) / ||G[n,:] - mean||.

Layout strategy per batch (all matmuls contract over the partition dim):
    X^T [64, N]  (8 PE transposes of DMA'd [128, 64] tiles)
    QKV^T = W^T-stacked lhsT [64, 96] @ X^T  -> [96, N] (+bias via ACT copy)
    S'_tile = (K^T chunk)^T @ Q^T chunk = S^T tile [128 m, 512 n]
    E^T = exp(S'/32)  (ACT, writes float32r)
    G^T += V_chunk^T-free lhsT [128 m, 32 e] @ E^T  (PSUM accumulation)
    G^T -> G natural (PE transposes), center+normalize rows (DVE/ACT)
    U natural -> U^T (PE transposes)
    corr tile = (U^T chunk)^T @ U^T chunk -> clip -> DMA out

Batch dim (64) is sharded across the 8 cores, params replicated.
"""

import sys

if "/opt/trn_rl_repo" not in sys.path:
    sys.path.insert(0, "/opt/trn_rl_repo")

import numpy as np

import concourse.bass as bass
import concourse.tile as tile
from concourse import mybir
from concourse.bass_utils import run_bass_kernel_spmd

F32 = mybir.dt.float32
F32R = mybir.dt.float32r
AF = mybir.ActivationFunctionType
ALU = mybir.AluOpType

N_CORES = 8
B = 64
N = 1024
D = 64
E = 32
P = 128
FREE = 512
NCHUNK = N // P  # 8
NF = N // FREE  # 2
B_PER_CORE = B // N_CORES  # 8


def split_multi_waits(nc):
    """The walrus build here accepts at most ONE sync wait per instruction
    ("Too many sync wait commands").  Hoist extra waits onto same-engine
    nops inserted immediately before the over-subscribed instruction."""
    ctr = 0
    for f in nc.m.functions:
        for bb in f.blocks:
            out = []
            for inst in bb.instructions:
                si = inst.sync_info
                if si is not None and si.on_wait and len(si.on_wait) > 1:
                    waits = list(si.on_wait)
                    for w in waits[:-1]:
                        ctr += 1
                        out.append(
                            mybir.InstNoOp(
                                name=f"I-ws{ctr}",
                                engine=inst.engine,
                                sync_info=mybir.SyncInfo(on_wait=[w], on_update=[]),
                            )
                        )
                    inst.sync_info = mybir.SyncInfo(
                        on_wait=[waits[-1]], on_update=list(si.on_update)
                    )
                out.append(inst)
            bb.instructions = out


DEFAULT_OPTS = dict(
    etp_bufs=2,
    otp_bufs=3,
    xin_bufs=3,
    copy_engine="vector",  # engine for vn/uT psum->sbuf copies
)


def build_nc(b_per_core=B_PER_CORE, repeat=1, **opts):
    o = {**DEFAULT_OPTS, **opts}
    nc = bass.Bass("TRN2", target_bir_lowering=False)
    X = nc.dram_tensor("X", [b_per_core, N, D], F32, kind="ExternalInput")
    WQKV = nc.dram_tensor("WQKV", [D, 3 * E], F32, kind="ExternalInput")
    BIA = nc.dram_tensor("BIA", [3 * E, 1], F32, kind="ExternalInput")
    IDN = nc.dram_tensor("IDN", [P, P], F32, kind="ExternalInput")
    OUT = nc.dram_tensor("OUT", [b_per_core, N, N], F32, kind="ExternalOutput")

    with tile.TileContext(nc) as tc:
        with (
            tc.tile_pool(name="const", bufs=1) as const,
            tc.tile_pool(name="sb", bufs=2) as sb,
            tc.tile_pool(name="xin", bufs=o["xin_bufs"]) as xin,
            tc.tile_pool(name="et", bufs=o["etp_bufs"]) as etp,
            tc.tile_pool(name="ot", bufs=o["otp_bufs"]) as otp,
            tc.tile_pool(name="small", bufs=3) as small,
            tc.tile_pool(name="pt", bufs=1, space="PSUM") as pt,
            tc.tile_pool(name="psum_u", bufs=2, space="PSUM") as ps_u,
            tc.tile_pool(name="psum_s", bufs=2, space="PSUM") as ps_s,
            tc.tile_pool(name="psum_c", bufs=2, space="PSUM") as ps_c,
            tc.tile_pool(name="psum_g", bufs=1, space="PSUM") as ps_g,
        ):
            # --- constants (replicated, loaded once) ---
            w_raw = const.tile([D, 3 * E], F32)
            nc.sync.dma_start(out=w_raw, in_=WQKV[:, :])
            wqkv = const.tile([D, 3 * E], F32R)
            nc.vector.tensor_copy(wqkv, w_raw)  # round to f32r
            biases = []
            for j in range(3):
                bt = const.tile([E, 1], F32, tag=f"bias{j}", name=f"bias{j}")
                nc.sync.dma_start(out=bt, in_=BIA[j * E : (j + 1) * E, :])
                biases.append(bt)
            idn = const.tile([P, P], F32)
            nc.sync.dma_start(out=idn, in_=IDN[:, :])
            idnr32 = const.tile([E, E], F32R)
            nc.vector.tensor_copy(idnr32, idn[0:E, 0:E])
            idnr128 = const.tile([P, P], F32R)
            nc.vector.tensor_copy(idnr128, idn)

            QUADS = NCHUNK // 4  # 2

            def st_front_steps(b):
                """Loads + X^T + projections + V natural; yields emit fns.
                Returns (state, steps_generator)."""
                st = {}

                def gen():
                    xT = sb.tile([D, N], F32R, tag="xT", name="xT")
                    qT = sb.tile([E, N], F32R, tag="qT", name="qT")
                    kT = sb.tile([E, N], F32R, tag="kT", name="kT")
                    vT = sb.tile([E, N], F32R, tag="vT", name="vT")
                    vn = sb.tile([P, NCHUNK, E], F32R, tag="vn", name="vn")
                    st.update(qT=qT, kT=kT, vn=vn)

                    def x_quad(q):
                        def emit():
                            px = pt.tile([D, 4 * P], F32, tag="t", name="px")
                            for j in range(4):
                                i = 4 * q + j
                                xn = xin.tile([P, D], F32, tag="xn", name="xn")
                                nc.scalar.dma_start(
                                    out=xn, in_=X[b, i * P : (i + 1) * P, :]
                                )
                                nc.tensor.transpose(
                                    px[:, j * P : (j + 1) * P], xn, idn
                                )
                            nc.vector.tensor_copy(
                                xT[:, q * 4 * P : (q + 1) * 4 * P], px
                            )

                        return emit

                    def proj(j, h, dst):
                        def emit():
                            pj = pt.tile([E, FREE], F32, tag="t", name="pj")
                            nc.tensor.matmul(
                                pj,
                                wqkv[:, j * E : (j + 1) * E],
                                xT[:, h * FREE : (h + 1) * FREE],
                                start=True,
                                stop=True,
                            )
                            nc.scalar.activation(
                                dst[:, h * FREE : (h + 1) * FREE],
                                pj,
                                AF.Identity,
                                bias=biases[j],
                                scale=1.0,
                            )

                        return emit

                    def v_quad(q):
                        def emit():
                            pv = pt.tile([P, 4 * E], F32R, tag="t", name="pv")
                            for j in range(4):
                                i = 4 * q + j
                                nc.tensor.transpose(
                                    pv[:, j * E : (j + 1) * E],
                                    vT[:, i * P : (i + 1) * P],
                                    idnr32,
                                )
                            nc.vector.tensor_copy(
                                vn[:, 4 * q : 4 * (q + 1), :], pv
                            )

                        return emit

                    for q in range(QUADS):
                        yield x_quad(q)
                    for j, dst in enumerate([qT, kT, vT]):
                        for h in range(NF):
                            yield proj(j, h, dst)
                    for q in range(QUADS):
                        yield v_quad(q)

                return st, gen()

            def st_sg_steps(b, st):
                """S' -> exp -> G^T accumulation; yields emit-callables."""
                qT, kT, vn = st["qT"], st["kT"], st["vn"]

                def prologue():
                    st["gT"] = sb.tile([E, N], F32, tag="gT", name="gT")
                    st["gp"] = None

                def step(h, i):
                    def emit():
                        if i == 0:
                            st["gp"] = ps_g.tile(
                                [E, FREE], F32, tag="g", name="gp"
                            )
                        pss = ps_s.tile([P, FREE], F32, tag="s", name="pss")
                        nc.tensor.matmul(
                            pss,
                            kT[:, i * P : (i + 1) * P],
                            qT[:, h * FREE : (h + 1) * FREE],
                            start=True,
                            stop=True,
                        )
                        et = etp.tile([P, FREE], F32R, tag="et", name="et")
                        nc.scalar.activation(et, pss, AF.Exp, scale=1.0 / 32.0)
                        nc.tensor.matmul(
                            st["gp"],
                            vn[:, i, :],
                            et,
                            start=(i == 0),
                            stop=(i == NCHUNK - 1),
                        )

                    return emit

                def gt_copy(h):
                    def emit():
                        nc.vector.tensor_copy(
                            st["gT"][:, h * FREE : (h + 1) * FREE], st["gp"]
                        )

                    return emit

                prologue()
                for h in range(NF):
                    for i in range(NCHUNK):
                        yield step(h, i)
                    yield gt_copy(h)

            def st_norm_steps(b, st):
                """Column-normalize G^T (already centered) -> U^T f32r."""

                def quad(q, uT):
                    def emit():
                        pg = ps_u.tile([P, 4 * E], F32, tag="u", name="pg")
                        for j in range(4):
                            i = 4 * q + j
                            nc.tensor.transpose(
                                pg[:, j * E : (j + 1) * E],
                                st["gT"][:, i * P : (i + 1) * P],
                                idn[0:E, 0:E],
                            )
                        sqg = small.tile([P, 4 * E], F32, tag="sqg", name="sqg")
                        nrm = small.tile([P, 4], F32, tag="nrm", name="nrm")
                        for j in range(4):
                            nc.scalar.activation(
                                sqg[:, j * E : (j + 1) * E],
                                pg[:, j * E : (j + 1) * E],
                                AF.Square,
                                accum_out=nrm[:, j : j + 1],
                            )
                        rfq = small.tile([P, 4], F32, tag="rfq", name="rfq")
                        nc.scalar.activation(rfq, nrm, AF.Sqrt)
                        rrq = small.tile([P, 4], F32, tag="rrq", name="rrq")
                        nc.vector.reciprocal(rrq, rfq)
                        unp = small.tile([P, 4 * E], F32R, tag="unp", name="unp")
                        for j in range(4):
                            nc.vector.tensor_scalar_mul(
                                unp[:, j * E : (j + 1) * E],
                                pg[:, j * E : (j + 1) * E],
                                rrq[:, j : j + 1],
                            )
                        pu = ps_u.tile([E, 4 * P], F32R, tag="u", name="pu")
                        for j in range(4):
                            nc.tensor.transpose(
                                pu[:, j * P : (j + 1) * P],
                                unp[:, j * E : (j + 1) * E],
                                idnr128,
                            )
                        nc.vector.tensor_copy(
                            uT[:, q * 4 * P : (q + 1) * 4 * P], pu
                        )

                    return emit

                uT = sb.tile([E, N], F32R, tag="uT", name="uT")
                st["uT"] = uT
                for q in range(QUADS):
                    yield quad(q, uT)

            def st_corr_steps(b, st):
                """corr = clip(U U^T) -> DRAM; yields emit-callables."""
                uT = st["uT"]

                def step(i):
                    def emit():
                        ot = otp.tile([P, N], F32, tag="ot", name="ot")
                        for h in range(NF):
                            pc = ps_c.tile(
                                [P, FREE], F32, tag="c", name="pc"
                            )
                            nc.tensor.matmul(
                                pc,
                                uT[:, i * P : (i + 1) * P],
                                uT[:, h * FREE : (h + 1) * FREE],
                                start=True,
                                stop=True,
                            )
                            nc.vector.tensor_scalar(
                                ot[:, h * FREE : (h + 1) * FREE],
                                pc,
                                1.0,
                                -1.0,
                                ALU.min,
                                ALU.max,
                            )
                        nc.sync.dma_start(
                            out=OUT[b, i * P : (i + 1) * P, :], in_=ot
                        )

                    return emit

                for i in range(NCHUNK):
                    yield step(i)

            def merge_emit(gen_a, gen_b, ratio=2):
                """Interleave emission: `ratio` steps of a per step of b."""
                a, bq = list(gen_a), list(gen_b)
                ia = ib = 0
                while ia < len(a) or ib < len(bq):
                    for _ in range(ratio):
                        if ia < len(a):
                            a[ia]()
                            ia += 1
                    if ib < len(bq):
                        bq[ib]()
                        ib += 1

            batches = [bb for _r in range(repeat) for bb in range(b_per_core)]
            st0, front_gen = st_front_steps(batches[0])
            for emit in front_gen:
                emit()
            front = st0
            prev = None  # (b, state) with uT pending corr
            for idx, b in enumerate(batches):
                cur = front
                sg = st_sg_steps(b, cur)
                corr_prev = (
                    st_corr_steps(prev[0], prev[1])
                    if prev is not None
                    else iter(())
                )
                merge_emit(sg, corr_prev, ratio=2)
                norm = st_norm_steps(b, cur)
                if idx + 1 < len(batches):
                    nxt_st, nxt_gen = st_front_steps(batches[idx + 1])
                    merge_emit(nxt_gen, norm, ratio=5)
                    front = nxt_st
                else:
                    for emit in norm:
                        emit()
                prev = (b, cur)
            for emit in st_corr_steps(prev[0], prev[1]):
                emit()

    split_multi_waits(nc)
    return nc


_NC_CACHE = {}


def _get_nc(b_per_core, repeat=1):
    key = (b_per_core, repeat)
    if key not in _NC_CACHE:
        _NC_CACHE[key] = build_nc(b_per_core, repeat)
    return _NC_CACHE[key]


def make_in_maps(BOLDSignals, Wq, bq, Wk, bk, Wv, bv, n_cores=N_CORES):
    # Fold the feature-centering of G into the V projection:
    # G = E @ (X Wv^T + bv) and centering G's rows over the E=32 features
    # is linear, so center Wv's output dim (and bv) on the host instead.
    Wq, bq = np.asarray(Wq, np.float64), np.asarray(bq, np.float64)
    Wk, bk = np.asarray(Wk, np.float64), np.asarray(bk, np.float64)
    Wv, bv = np.asarray(Wv, np.float64), np.asarray(bv, np.float64)
    Wv_c = Wv - Wv.mean(axis=0, keepdims=True)
    bv_c = bv - bv.mean()
    wqkv = np.concatenate([Wq.T, Wk.T, Wv_c.T], axis=1).astype(np.float32)
    bia = np.concatenate([bq, bk, bv_c]).astype(np.float32)[:, None]
    idn = np.eye(P, dtype=np.float32)
    b_per_core = BOLDSignals.shape[0] // n_cores
    in_maps = []
    for c in range(n_cores):
        in_maps.append(
            {
                "X": np.ascontiguousarray(
                    BOLDSignals[c * b_per_core : (c + 1) * b_per_core],
                    dtype=np.float32,
                ),
                "WQKV": wqkv,
                "BIA": bia,
                "IDN": idn,
            }
        )
    return in_maps


def kernel(
    BOLDSignals,
    EmptyCorrelations=None,
    Wq=None,
    bq=None,
    Wk=None,
    bk=None,
    Wv=None,
    bv=None,
    **_unused,
):
    BOLDSignals = np.asarray(BOLDSignals, dtype=np.float32)
    nb = BOLDSignals.shape[0]
    assert nb % N_CORES == 0, nb
    b_per_core = nb // N_CORES
    nc = _get_nc(b_per_core)
    in_maps = make_in_maps(BOLDSignals, Wq, bq, Wk, bk, Wv, bv)
    res = run_bass_kernel_spmd(nc, in_maps, core_ids=list(range(N_CORES)))
    return np.concatenate([res.results[c]["OUT"] for c in range(N_CORES)], axis=0)


if __name__ == "__main__":
    rng = np.random.default_rng(0)
    inputs = {
        "BOLDSignals": rng.standard_normal((B, N, D), dtype=np.float32),
        "EmptyCorrelations": np.zeros((B, N, N), dtype=np.float32),
    }
    bound = 1.0 / np.sqrt(D)
    for nm in ["q", "k", "v"]:
        inputs[f"W{nm}"] = rng.uniform(-bound, bound, (E, D)).astype(np.float32)
        inputs[f"b{nm}"] = rng.uniform(-bound, bound, (E,)).astype(np.float32)
    out = kernel(**inputs)
    print("out", out.shape, out.dtype, out.min(), out.max())
